# revision 1
# baseline (speedup 1.0000x reference)
"""Trainium2 Bass kernel for nn_Encoder_Head_77343771066713 (DGCNN+PCT encoder).

Data-parallel over batch B=8 across 8 NeuronCores (one point cloud per core).
Self-contained: hardcodes all shapes. kernel(**inputs) -> (8, 256, 2048) f32.

Per-core pipeline (N=2048 points, K=40 neighbors):
  kNN1 on xyz -> graph-gather -> conv1/conv2 (P/Q decomposition) -> max_k -> x1
  kNN2 on x1  -> PP-gather    -> conv3/conv4 (P/Q)               -> max_k -> x2
  h=[x1;x2] -> 4x offset self-attention (fused softmax, no row-max) -> fuse conv

Top-40 selection: low-6-bit mantissa index encode + per-64-chunk max8 +
exact phase-2 (max8/max_index/match_replace) + arithmetic global-index decode.
"""
import numpy as np

N = 2048
K = 40
B = 8
NCORES = 8
NT = N // 128          # 16 row tiles
EPS_BN = 1e-5

_COMPILED = None


def _build_program(use_f32r=False):
    import concourse.bass as bass
    import concourse.tile as tile
    from concourse import bacc, mybir

    f32 = mybir.dt.float32
    f32r = mybir.dt.float32r if use_f32r else mybir.dt.float32
    f32R = mybir.dt.float32r
    u32 = mybir.dt.uint32
    u16 = mybir.dt.uint16
    i16 = mybir.dt.int16
    AF = mybir.ActivationFunctionType
    OP = mybir.AluOpType
    AX = mybir.AxisListType
    ts = bass.ts

    nc = bacc.Bacc("TRN2", target_bir_lowering=False, debug=False)

    def din(name, shape, dt=f32):
        return nc.dram_tensor(name, shape, dt, kind="ExternalInput")

    xt16 = din("xt16", [16, N])
    colenc = din("colenc", [128, N], u32)
    cst = din("cst", [128, 8], u32)
    w1a_q = din("w1a_q", [128, 256])
    w1q_T = din("w1q_T", [4, 64])
    w2_T = din("w2_T", [128, 64])
    bn2_s = din("bn2_s", [128, 1])
    bn2_b = din("bn2_b", [128, 1])
    w3a_T = din("w3a_T", [65, 64])
    w3q_T = din("w3q_T", [65, 64])
    w4_T = din("w4_T", [128, 64])
    bn4_s = din("bn4_s", [128, 1])
    bn4_b = din("bn4_b", [128, 1])
    qk_T = din("qk_T", [4, 128, 32])
    v_Tr = din("v_Tr", [4, 128, 128])
    vb_rep = din("vb_rep", [4, 128, 128])
    t_T = din("t_T", [4, 128, 128])
    tb_f = din("tb_f", [4, 128, 1])
    fuse_T = din("fuse_T", [512, 256])
    y_out = nc.dram_tensor("y", [256, N], f32, kind="ExternalOutput")

    with tile.TileContext(nc) as tc:
      with tc.tile_pool(name="consts", bufs=1) as consts, \
           tc.tile_pool(name="big", bufs=1) as big:
        xt_s = consts.tile([16, N], f32)
        nc.sync.dma_start(xt_s[:], xt16[:])
        colenc_s = consts.tile([128, N], u32)
        nc.sync.dma_start(colenc_s[:], colenc[:])
        cst_s = consts.tile([128, 8], u32)
        nc.sync.dma_start(cst_s[:], cst[:])
        MASKC = cst_s[:, 0:1]     # 0xFFFFFFC0
        C63 = cst_s[:, 1:2]       # 63
        CFFF8 = cst_s[:, 2:3]     # 0xFFF8
        C8 = cst_s[:, 3:4]        # 8

        w1a_s = consts.tile([128, 256], f32)
        nc.sync.dma_start(w1a_s[:], w1a_q[:])
        w1q_s = consts.tile([4, 64], f32)
        nc.sync.dma_start(w1q_s[:], w1q_T[:])
        w2_s = consts.tile([128, 64], f32)
        nc.sync.dma_start(w2_s[:], w2_T[:])
        bn2s_s = consts.tile([128, 1], f32)
        nc.sync.dma_start(bn2s_s[:], bn2_s[:])
        bn2b_s = consts.tile([128, 1], f32)
        nc.sync.dma_start(bn2b_s[:], bn2_b[:])
        w3a_s = consts.tile([65, 64], f32)
        nc.sync.dma_start(w3a_s[:], w3a_T[:])
        w3q_s = consts.tile([65, 64], f32)
        nc.sync.dma_start(w3q_s[:], w3q_T[:])
        w4_s = consts.tile([128, 64], f32)
        nc.sync.dma_start(w4_s[:], w4_T[:])
        bn4s_s = consts.tile([128, 1], f32)
        nc.sync.dma_start(bn4s_s[:], bn4_s[:])
        bn4b_s = consts.tile([128, 1], f32)
        nc.sync.dma_start(bn4b_s[:], bn4_b[:])
        fuse_s = consts.tile([128, 1024], f32)
        for k in range(4):
            nc.sync.dma_start(fuse_s[:, 256 * k:256 * k + 256],
                              fuse_T[128 * k:128 * k + 128, :])
        ones31 = consts.tile([3, 1], f32)
        nc.vector.memset(ones31[:], 1.0)
        ones64 = consts.tile([64, 1], f32)
        nc.vector.memset(ones64[:], 1.0)
        ones1 = consts.tile([128, 128], f32)
        nc.vector.memset(ones1[:], 1.0)
        ones_row = consts.tile([1, N], f32)
        nc.vector.memset(ones_row[:], 1.0)

        # long-lived tensors
        idx1_w = big.tile([128, 640], i16, tag="idx1w")
        idx2_w = big.tile([128, 2560], i16, tag="idx2w")
        x1_stack = big.tile([128, 1024], f32, tag="x1st")
        x2_stack = big.tile([128, 1024], f32, tag="x2st")
        hs = [big.tile([128, N], f32, name=f"h{i}", tag=f"h{i}")
              for i in range(5)]

        # ---------------- top-k machinery ----------------
        def topk_tiles(Amat, Bmat, idx_w, layer, tile_order):
            with tc.tile_pool(name=f"sp{layer}", bufs=2, space="PSUM") as spp, \
                 tc.tile_pool(name=f"se{layer}", bufs=2) as sep, \
                 tc.tile_pool(name=f"svp{layer}", bufs=2) as svp:
                for t in tile_order:
                    spt = spp.tile([128, N], f32, tag="spt")
                    lhsT = Amat[:, t * 128:(t + 1) * 128]
                    for c in range(4):
                        nc.tensor.matmul(spt[:, ts(c, 512)], lhsT,
                                         Bmat[:, ts(c, 512)],
                                         start=True, stop=True)
                    senc = sep.tile([128, N], f32, tag="senc")
                    nc.vector.tensor_tensor(
                        out=senc[:].bitcast(u32), in0=spt[:].bitcast(u32),
                        in1=MASKC.broadcast_to((128, N)),
                        op=OP.bitwise_and)
                    nc.vector.tensor_tensor(
                        out=senc[:].bitcast(u32), in0=senc[:].bitcast(u32),
                        in1=colenc_s[:], op=OP.bitwise_or)
                    sv = svp.tile([128, 256], f32, tag="sv")
                    for ch in range(32):
                        nc.vector.max(sv[:, 8 * ch:8 * ch + 8],
                                      senc[:, 64 * ch:64 * ch + 64])
                    m8 = svp.tile([128, 40], f32, tag="m8")
                    pos = svp.tile([128, 40], u32, tag="pos")
                    wk0 = svp.tile([128, 256], f32, tag="wk0")
                    wk1 = svp.tile([128, 256], f32, tag="wk1")
                    cur = sv
                    for r in range(5):
                        nc.vector.max(m8[:, 8 * r:8 * r + 8], cur[:])
                        nc.vector.max_index(pos[:, 8 * r:8 * r + 8],
                                            m8[:, 8 * r:8 * r + 8], cur[:])
                        if r < 4:
                            nxt = wk0 if r % 2 == 0 else wk1
                            nc.vector.match_replace(nxt[:],
                                                    m8[:, 8 * r:8 * r + 8],
                                                    cur[:], -3.0e38)
                            cur = nxt
                    loc = svp.tile([128, 40], u32, tag="loc")
                    nc.vector.tensor_tensor(out=loc[:],
                                            in0=m8[:].bitcast(u32),
                                            in1=C63.broadcast_to((128, 40)),
                                            op=OP.bitwise_and)
                    chb = svp.tile([128, 40], u32, tag="chb")
                    nc.vector.tensor_tensor(out=chb[:], in0=pos[:],
                                            in1=CFFF8.broadcast_to((128, 40)),
                                            op=OP.bitwise_and)
                    gid = svp.tile([128, 40], u32, tag="gid")
                    nc.vector.tensor_tensor(out=gid[:], in0=chb[:],
                                            in1=C8.broadcast_to((128, 40)),
                                            op=OP.mult)
                    nc.vector.tensor_tensor(out=gid[:], in0=gid[:],
                                            in1=loc[:], op=OP.add)
                    stg = svp.tile([128, 40], u16, tag="stg")
                    nc.vector.tensor_copy(stg[:], gid[:])
                    if layer == 1:
                        g, tt = t // 2, t % 2
                        for bb in range(8):
                            beta = 8 * tt + bb
                            nc.sync.dma_start(
                                idx_w[16 * g:16 * g + 16,
                                      40 * beta:40 * beta + 40],
                                stg[16 * bb:16 * bb + 16, :].bitcast(i16))
                    else:
                        h, tt = t // 8, t % 8
                        for bb in range(8):
                            beta = 8 * tt + bb
                            nc.sync.dma_start(
                                idx_w[64 * h:64 * h + 16,
                                      40 * beta:40 * beta + 40],
                                stg[16 * bb:16 * bb + 16, :].bitcast(i16))
                        for q in range(1, 4):
                            nc.sync.dma_start(
                                idx_w[64 * h + 16 * q:64 * h + 16 * q + 16,
                                      320 * tt:320 * tt + 320],
                                idx_w[64 * h:64 * h + 16,
                                      320 * tt:320 * tt + 320])

        # ---------------- shared conv block ----------------
        def conv_block(gath, g_or_none, qqr, n0_col, w_first, wsec, bns, bnb,
                       xstack, layer, cpool, ppool):
            for c in range(4):
                fs = slice(1280 * c, 1280 * (c + 1))
                if layer == 1:
                    q = g_or_none
                    b = q // 2
                    mm = ppool.tile([128, 1280], f32, tag="cv1")
                    rhs = gath[64 * b:64 * b + 64, fs]
                    for half in range(2):
                        v = 2 * (q % 2) + half
                        lhsT = w_first[64 * b:64 * b + 64,
                                       64 * v:64 * v + 64]
                        for m in range(3):
                            w = 512 if m < 2 else 256
                            nc.tensor.matmul(
                                mm[64 * half:64 * half + 64,
                                   512 * m:512 * m + w],
                                lhsT.bitcast(f32r),
                                rhs[:, 512 * m:512 * m + w].bitcast(f32r),
                                start=True, stop=True)
                    src0 = mm
                else:
                    src0 = gath[:, fs]
                summ = cpool.tile([128, 1280], f32, tag="sum")
                qin = qqr[:, n0_col + 32 * c:n0_col + 32 * c + 32]
                qap = qin.rearrange("p (b u w) -> p b u w", u=1, w=16)
                qap = qap.broadcast_to((128, 2, 40, 16))
                nc.vector.tensor_tensor(
                    out=summ[:].rearrange("p (b j w) -> p b j w", j=40, w=16),
                    in0=(src0[:] if layer == 1 else src0).rearrange(
                        "p (b j w) -> p b j w", j=40, w=16),
                    in1=qap, op=OP.add)
                lr1 = cpool.tile([128, 1280], f32, tag="lr1")
                nc.vector.scalar_tensor_tensor(out=lr1[:], in0=summ[:],
                                               scalar=0.2, in1=summ[:],
                                               op0=OP.mult, op1=OP.max)
                mm2 = ppool.tile([128, 1280], f32, tag="cv2")
                for half in range(2):
                    rhs = lr1[64 * half:64 * half + 64, :]
                    wse = wsec[64 * half:64 * half + 64, :]
                    for m in range(3):
                        w = 512 if m < 2 else 256
                        nc.tensor.matmul(
                            mm2[64 * half:64 * half + 64, 512 * m:512 * m + w],
                            wse.bitcast(f32r),
                            rhs[:, 512 * m:512 * m + w].bitcast(f32r),
                            start=True, stop=True)
                c2a = cpool.tile([128, 1280], f32, tag="c2a")
                nc.scalar.activation(c2a[:], mm2[:], AF.Identity,
                                     bias=bnb, scale=bns)
                c2 = cpool.tile([128, 1280], f32, tag="c2")
                nc.vector.scalar_tensor_tensor(out=c2[:], in0=c2a[:],
                                               scalar=0.2, in1=c2a[:],
                                               op0=OP.mult, op1=OP.max)
                red = c2[:].rearrange("p (b j w) -> p b w j", j=40, w=16)
                nc.vector.tensor_reduce(
                    out=xstack[:, n0_col + 32 * c:n0_col + 32 * c + 32],
                    in_=red, axis=AX.X, op=OP.max)

        # ================= kNN1 =================
        with tc.tile_pool(name="kn1", bufs=1) as kn1:
            A1 = kn1.tile([34, N], f32, tag="A1")
            B1 = kn1.tile([34, N], f32, tag="B1")
            sq = kn1.tile([3, N], f32, tag="sq1")
            nc.vector.memset(A1[:], 0.0)
            nc.vector.memset(B1[:], 0.0)
            nc.scalar.activation(sq[:], xt_s[0:3, :], AF.Square)
            with tc.tile_pool(name="xxp", bufs=1, space="PSUM") as xxp:
                xxpt = xxp.tile([34, N], f32)
                for c in range(4):
                    nc.tensor.matmul(xxpt[32:33, ts(c, 512)], ones31[:],
                                     sq[:, ts(c, 512)], start=True, stop=True)
                nc.scalar.activation(A1[0:3, :], xt_s[0:3, :], AF.Copy,
                                     scale=2.0)
                nc.scalar.activation(A1[32:33, :], xxpt[32:33, :], AF.Copy,
                                     scale=-1.0)
                nc.sync.dma_start(A1[33:34, :], ones_row[:])
                nc.scalar.activation(B1[0:3, :], xt_s[0:3, :], AF.Copy)
                nc.vector.memset(B1[32:33, :], 1.0)
                nc.sync.dma_start(B1[33:34, :], A1[32:33, :])
            order1 = [2 * g + tt for tt in range(2) for g in range(8)]
            topk_tiles(A1, B1, idx1_w, 1, order1)

        # ================= layer 1 =================
        with tc.tile_pool(name="ly1", bufs=1) as ly1:
            xt_rep = ly1.tile([128, N], f32, tag="xtr")
            for g in range(8):
                nc.sync.dma_start(xt_rep[16 * g:16 * g + 16, :], xt_s[:])
            aug4 = ly1.tile([4, N], f32, tag="aug4")
            nc.scalar.activation(aug4[0:3, :], xt_s[0:3, :], AF.Copy)
            nc.sync.dma_start(aug4[3:4, :], ones_row[:])
            QQ1r = ly1.tile([128, 1024], f32, tag="QQ1r")
            with tc.tile_pool(name="qq1p", bufs=1, space="PSUM") as qq1p:
                qq1t = qq1p.tile([64, N], f32)
                for c in range(4):
                    nc.tensor.matmul(qq1t[:, ts(c, 512)], w1q_s[:],
                                     aug4[:, ts(c, 512)], start=True,
                                     stop=True)
                qq1f = ly1.tile([64, N], f32, tag="qq1f")
                nc.scalar.activation(qq1f[:], qq1t[:], AF.Copy)
                for q in range(4):
                    nc.sync.dma_start(QQ1r[0:64, 256 * q:256 * q + 256],
                                      qq1f[:, 512 * q:512 * q + 256])
                    nc.sync.dma_start(QQ1r[64:128, 256 * q:256 * q + 256],
                                      qq1f[:, 512 * q + 256:512 * q + 512])
            with tc.tile_pool(name="g1", bufs=1) as g1p, \
                 tc.tile_pool(name="cv1s", bufs=2) as cv1s, \
                 tc.tile_pool(name="cv1p", bufs=1, space="PSUM") as cv1p:
                for tt in range(2):
                    g1 = g1p.tile([128, 5120], f32, tag="g1")
                    nc.gpsimd.ap_gather(
                        out_ap=g1[:], in_ap=xt_rep[:],
                        idxs_ap=idx1_w[:, 320 * tt:320 * tt + 320],
                        channels=128, num_elems=N, d=1, num_idxs=5120)
                    for q in range(4):
                        conv_block(g1, q, QQ1r, 256 * q + 128 * tt,
                                   w1a_s, w2_s, bn2s_s[:], bn2b_s[:],
                                   x1_stack, 1, cv1s, cv1p)

        h0 = hs[0]
        for q in range(4):
            nc.sync.dma_start(h0[0:64, 512 * q:512 * q + 256],
                              x1_stack[0:64, 256 * q:256 * q + 256])
            nc.sync.dma_start(h0[0:64, 512 * q + 256:512 * q + 512],
                              x1_stack[64:128, 256 * q:256 * q + 256])

        # PP/QQ2 (outlive kNN2 scope)
        with tc.tile_pool(name="ly2pre", bufs=1) as ly2pre:
            PPrep = ly2pre.tile([128, N], f32, tag="PPrep")
            QQ2r = ly2pre.tile([128, 1024], f32, tag="QQ2r")
            x1aug = ly2pre.tile([65, N], f32, tag="x1aug")
            nc.sync.dma_start(x1aug[0:64, :], h0[0:64, :])
            nc.sync.dma_start(x1aug[64:65, :], ones_row[:])
            with tc.tile_pool(name="ppp", bufs=1, space="PSUM") as ppp:
                ppt = ppp.tile([64, N], f32, tag="pp")
                for c in range(4):
                    nc.tensor.matmul(ppt[:, ts(c, 512)],
                                     w3a_s[:].bitcast(f32r),
                                     x1aug[:, ts(c, 512)].bitcast(f32r),
                                     start=True, stop=True)
                nc.scalar.activation(PPrep[0:64, :], ppt[:], AF.Copy)
                nc.sync.dma_start(PPrep[64:128, :], PPrep[0:64, :])
                qq2t = ppp.tile([64, N], f32, tag="qq2")
                for c in range(4):
                    nc.tensor.matmul(qq2t[:, ts(c, 512)],
                                     w3q_s[:].bitcast(f32r),
                                     x1aug[:, ts(c, 512)].bitcast(f32r),
                                     start=True, stop=True)
                qq2f = ly2pre.tile([64, N], f32, tag="qq2f")
                nc.scalar.activation(qq2f[:], qq2t[:], AF.Copy)
                nc.sync.dma_start(QQ2r[0:64, :], qq2f[:, 0:1024])
                nc.sync.dma_start(QQ2r[64:128, :], qq2f[:, 1024:2048])

            # ================= kNN2 =================
            with tc.tile_pool(name="kn2", bufs=1) as kn2:
                A2 = kn2.tile([66, N], f32, tag="A2")
                B2 = kn2.tile([66, N], f32, tag="B2")
                sq2 = kn2.tile([64, N], f32, tag="sq2")
                nc.sync.dma_start(B2[0:64, :], h0[0:64, :])
                nc.scalar.activation(A2[0:64, :], B2[0:64, :], AF.Copy,
                                     scale=2.0)
                nc.scalar.activation(sq2[:], B2[0:64, :], AF.Square)
                with tc.tile_pool(name="xx2p", bufs=1, space="PSUM") as xx2p:
                    xx2t = xx2p.tile([66, N], f32)
                    for c in range(4):
                        nc.tensor.matmul(xx2t[64:65, ts(c, 512)], ones64[:],
                                         sq2[:, ts(c, 512)], start=True,
                                         stop=True)
                    nc.scalar.activation(A2[64:65, :], xx2t[64:65, :],
                                         AF.Copy, scale=-1.0)
                    nc.sync.dma_start(A2[65:66, :], ones_row[:])
                    nc.vector.memset(B2[64:65, :], 1.0)
                    nc.sync.dma_start(B2[65:66, :], A2[64:65, :])
                order2 = [8 * h + tt for tt in range(8) for h in range(2)]
                topk_tiles(A2, B2, idx2_w, 2, order2)

            # ================= layer 2 =================
            with tc.tile_pool(name="g2", bufs=1) as g2p, \
                 tc.tile_pool(name="cv2s", bufs=2) as cv2s, \
                 tc.tile_pool(name="cv2p", bufs=1, space="PSUM") as cv2p:
                for tt in range(8):
                    g2 = g2p.tile([128, 5120], f32, tag="g2")
                    nc.gpsimd.ap_gather(
                        out_ap=g2[:], in_ap=PPrep[:],
                        idxs_ap=idx2_w[:, 320 * tt:320 * tt + 320],
                        channels=128, num_elems=N, d=1, num_idxs=5120)
                    conv_block(g2, None, QQ2r, 128 * tt, None, w4_s,
                               bn4s_s[:], bn4b_s[:], x2_stack, 2, cv2s, cv2p)

        nc.sync.dma_start(h0[64:128, 0:1024], x2_stack[0:64, :])
        nc.sync.dma_start(h0[64:128, 1024:2048], x2_stack[64:128, :])

        # ================= SA layers =================
        with tc.tile_pool(name="saw", bufs=1) as saw, \
             tc.tile_pool(name="sas", bufs=2) as sas:
            for l in range(4):
                h = hs[l]
                hn = hs[l + 1]
                qk_s = saw.tile([128, 32], f32, tag="qk")
                nc.sync.dma_start(qk_s[:], qk_T[l])
                v_s = saw.tile([128, 128], f32, tag="vs")
                nc.sync.dma_start(v_s[:], v_Tr[l])
                vb_s = saw.tile([128, 128], f32, tag="vb")
                nc.sync.dma_start(vb_s[:], vb_rep[l])
                tT_s = saw.tile([128, 128], f32, tag="tT")
                nc.sync.dma_start(tT_s[:], t_T[l])
                tb_s = saw.tile([128, 1], f32, tag="tb")
                nc.sync.dma_start(tb_s[:], tb_f[l])
                XQ = saw.tile([32, N], f32R, tag="XQ")
                xvT = saw.tile([128, N], f32, tag="xvT")

                with tc.tile_pool(name=f"sap{l}", bufs=2, space="PSUM") as sap:
                    xqp = sap.tile([32, N], f32, tag="xq")
                    for c in range(4):
                        nc.tensor.matmul(xqp[:, ts(c, 512)],
                                         qk_s[:].bitcast(f32r),
                                         h[:, ts(c, 512)].bitcast(f32r),
                                         start=True, stop=True)
                    nc.scalar.activation(XQ[:], xqp[:], AF.Copy)
                with tc.tile_pool(name=f"sav{l}", bufs=2,
                                  space="PSUM") as sav:
                    for t in range(NT):
                        xvp = sav.tile([128, 128], f32, tag="xv")
                        nc.tensor.matmul(xvp[:],
                                         h[:, ts(t, 128)].bitcast(f32r),
                                         v_s[:].bitcast(f32r),
                                         start=True, stop=True)
                        nc.vector.tensor_tensor(out=xvT[:, ts(t, 128)],
                                                in0=xvp[:], in1=vb_s[:],
                                                op=OP.add)

                with tc.tile_pool(name=f"sax{l}", bufs=1,
                                  space="PSUM") as sax:
                    xrp = sax.tile([128, N], f32, tag="xr")
                    csA = sax.tile([64, 512], f32, tag="csA")
                    csB = sax.tile([64, 512], f32, tag="csB")
                    cst4 = [csA[0:1, :], csA[32:33, :],
                            csB[0:1, :], csB[32:33, :]]
                    csr4 = cst4
                    with tc.tile_pool(name=f"sae{l}", bufs=2,
                                      space="PSUM") as sae:
                        for t in range(NT):
                            ext = sas.tile([128, N], f32R, tag="ex")
                            rs4 = sas.tile([128, 4], f32, tag="rs4")
                            for c in range(4):
                                ep = sae.tile([128, 512], f32, tag="ep")
                                nc.tensor.matmul(
                                    ep[:], XQ[:, ts(t, 128)],
                                    XQ[:, ts(c, 512)],
                                    start=True, stop=True)
                                nc.scalar.activation(
                                    ext[:, ts(c, 512)], ep[:], AF.Exp,
                                    accum_out=rs4[:, c:c + 1])
                            rsum = sas.tile([128, 1], f32, tag="rsum")
                            nc.vector.tensor_reduce(out=rsum[:], in_=rs4[:],
                                                    axis=AX.X, op=OP.add)
                            invr = sas.tile([128, 1], f32, tag="invr")
                            nc.vector.reciprocal(invr[:], rsum[:])
                            xvl = sas.tile([128, 128], f32R, tag="xvl")
                            nc.scalar.activation(xvl[:], xvT[:, ts(t, 128)],
                                                 AF.Copy, scale=invr[:])

                            for c in range(4):
                                nc.tensor.matmul(
                                    cst4[c], invr[:],
                                    ext[:, ts(c, 512)].bitcast(f32),
                                    start=(t == 0), stop=(t == NT - 1))
                            for c in range(4):
                                nc.tensor.matmul(
                                    xrp[:, ts(c, 512)], xvl[:],
                                    ext[:, ts(c, 512)],
                                    start=(t == 0), stop=(t == NT - 1))
                    # normalize + residual
                    rcs = sas.tile([64, 1024], f32, tag="rcs")
                    rcs4 = [rcs[0:1, 0:512], rcs[32:33, 0:512],
                            rcs[0:1, 512:1024], rcs[32:33, 512:1024]]
                    for c in range(4):
                        nc.vector.tensor_scalar(
                            out=rcs4[c], in0=csr4[c],
                            scalar1=1e-9, scalar2=None, op0=OP.add)
                        nc.vector.reciprocal(rcs4[c], rcs4[c])
                    d = sas.tile([128, N], f32, tag="d")
                    with tc.tile_pool(name=f"sab{l}", bufs=2,
                                      space="PSUM") as sab:
                        for c in range(4):
                            base = 32 * (c % 2)
                            rbp = sab.tile([128, 512], f32, tag="rb")
                            nc.tensor.matmul(rbp[:],
                                             ones1[base:base + 1, :],
                                             rcs4[c],
                                             start=True, stop=True)
                            rbs = sas.tile([128, 512], f32, tag="rbs")
                            nc.scalar.activation(rbs[:], rbp[:], AF.Copy)
                            xrn = sas.tile([128, 512], f32, tag="xrn")
                            nc.vector.tensor_tensor(out=xrn[:],
                                                    in0=xrp[:, ts(c, 512)],
                                                    in1=rbs[:], op=OP.mult)
                            nc.vector.tensor_tensor(out=d[:, ts(c, 512)],
                                                    in0=h[:, ts(c, 512)],
                                                    in1=xrn[:],
                                                    op=OP.subtract)
                with tc.tile_pool(name=f"sad{l}", bufs=2, space="PSUM") as sad:
                    for c in range(4):
                        drp = sad.tile([128, 512], f32, tag="dr")
                        nc.tensor.matmul(drp[:], tT_s[:].bitcast(f32r),
                                         d[:, ts(c, 512)].bitcast(f32r),
                                         start=True, stop=True)
                        rl = sas.tile([128, 512], f32, tag="rl")
                        nc.scalar.activation(rl[:], drp[:], AF.Relu,
                                             bias=tb_s[:], scale=1.0)
                        nc.vector.tensor_tensor(out=hn[:, ts(c, 512)],
                                                in0=h[:, ts(c, 512)],
                                                in1=rl[:], op=OP.add)

        # ================= fuse =================
        with tc.tile_pool(name="fup", bufs=2, space="PSUM") as fup, \
             tc.tile_pool(name="fus", bufs=2) as fus:
            for o in range(2):
                for c in range(4):
                    fp = fup.tile([128, 512], f32, tag="fp")
                    for k in range(4):
                        nc.tensor.matmul(
                            fp[:],
                            fuse_s[:, 256 * k + 128 * o:
                                   256 * k + 128 * o + 128].bitcast(f32r),
                            hs[k + 1][:, ts(c, 512)].bitcast(f32r),
                            start=(k == 0), stop=(k == 3))
                    yta = fus.tile([128, 512], f32, tag="yta")
                    nc.scalar.activation(yta[:], fp[:], AF.Copy)
                    yt = fus.tile([128, 512], f32, tag="yt")
                    nc.vector.scalar_tensor_tensor(out=yt[:], in0=yta[:],
                                                   scalar=0.2, in1=yta[:],
                                                   op0=OP.mult, op1=OP.max)
                    nc.sync.dma_start(y_out[128 * o:128 * o + 128, ts(c, 512)],
                                      yt[:])

    nc.compile()
    return nc


def _w1a_quad(w1a):
    """(64,3) folded conv1 weight -> (128,256) window-replicated lhsT with 4
    group-select variants (v-th variant active at local rows 16v..16v+2)."""
    pat = np.zeros((64, 256), np.float32)
    for v in range(4):
        pat[16 * v:16 * v + 3, 64 * v:64 * v + 64] = w1a.T
    return np.ascontiguousarray(np.tile(pat, (2, 1)))


def _prep_inputs(inputs):
    """Host-side: fold BN into weights, build per-core in_maps."""
    f = lambda a: np.ascontiguousarray(np.asarray(a, np.float32))
    x = f(inputs["x"])                      # (8, 2048, 3)
    scale = lambda g: (np.asarray(g, np.float32)
                       / np.sqrt(np.float32(1.0 + EPS_BN)))

    s1, b1 = scale(inputs["g1"]), f(inputs["b1"])
    w1 = f(inputs["w1"])
    w1a = s1[:, None] * w1[:, 0:3]
    w1q = np.concatenate([s1[:, None] * (w1[:, 3:6] - w1[:, 0:3]),
                          b1[:, None]], axis=1)          # (64, 4)
    s2v, b2v = scale(inputs["g2"]), f(inputs["b2"])
    s3, b3 = scale(inputs["g3"]), f(inputs["b3"])
    w3 = f(inputs["w3"])
    w3a = np.concatenate([s3[:, None] * w3[:, 0:64],
                          np.zeros((64, 1), np.float32)], axis=1)  # (64, 65)
    w3q = np.concatenate([s3[:, None] * (w3[:, 64:128] - w3[:, 0:64]),
                          b3[:, None]], axis=1)          # (64, 65)
    s4v, b4v = scale(inputs["g4"]), f(inputs["b4"])

    qk = f(inputs["sa_qk"])                 # (4, 32, 128)
    v = f(inputs["sa_v"])                   # (4, 128, 128)
    vb = f(inputs["sa_vb"])                 # (4, 128)
    t = f(inputs["sa_t"])
    tb = f(inputs["sa_tb"])
    sg = scale(inputs["sa_g"])              # (4, 128)
    sb = f(inputs["sa_b"])
    fuse_w = f(inputs["fuse_w"])
    sf = scale(inputs["fuse_g"])

    common = {
        "colenc": np.tile(np.arange(64, dtype=np.uint32), N // 64)[None, :]
                    .repeat(128, 0).copy(),
        "cst": np.array([0xFFFFFFC0, 63, 0xFFF8, 8, 0, 0, 0, 0],
                        dtype=np.uint32)[None, :].repeat(128, 0).copy(),
        "w1a_q": _w1a_quad(w1a),
        "w1q_T": np.ascontiguousarray(w1q.T),
        "w2_T": np.ascontiguousarray(
            np.tile(f(inputs["w2"]).T, (2, 1))),
        "bn2_s": np.tile(s2v, 2)[:, None].copy(),
        "bn2_b": np.tile(b2v, 2)[:, None].copy(),
        "w3a_T": np.ascontiguousarray(w3a.T),
        "w3q_T": np.ascontiguousarray(w3q.T),
        "w4_T": np.ascontiguousarray(
            np.tile(f(inputs["w4"]).T, (2, 1))),
        "bn4_s": np.tile(s4v, 2)[:, None].copy(),
        "bn4_b": np.tile(b4v, 2)[:, None].copy(),
        "qk_T": np.ascontiguousarray(np.transpose(qk, (0, 2, 1))),
        "v_Tr": np.ascontiguousarray(np.transpose(v, (0, 2, 1))),
        "vb_rep": np.ascontiguousarray(vb[:, None, :].repeat(128, 1)),
        "t_T": np.ascontiguousarray(
            np.transpose(sg[:, :, None] * t, (0, 2, 1))),
        "tb_f": np.ascontiguousarray((sg * tb + sb)[:, :, None]),
        "fuse_T": np.ascontiguousarray((sf[:, None] * fuse_w).T),
    }
    in_maps = []
    for b in range(B):
        xt = np.zeros((16, N), np.float32)
        xt[0:3, :] = x[b].T
        m = dict(common)
        m["xt16"] = xt
        in_maps.append(m)
    return in_maps


def kernel(**inputs):
    global _COMPILED
    from concourse.bass_utils import run_bass_kernel_spmd
    if _COMPILED is None:
        _COMPILED = _build_program()
    nc = _COMPILED
    in_maps = _prep_inputs(inputs)
    res = run_bass_kernel_spmd(nc, in_maps, list(range(NCORES)))
    out = np.stack([res.results[b]["y"] for b in range(B)], axis=0)
    return out.astype(np.float32)

